# revision 13
# baseline (speedup 1.0000x reference)
"""Causal self-attention with relative position (skew trick), 8-way
head-sharded across NeuronCores.

Shapes (hardcoded): x [4, 2048, 1024], W_attn [1024, 3072], b_attn [3072],
Er [2048, 64], W_proj [1024, 1024], b_proj [1024].  16 heads of 64; each of
the 8 cores handles 2 heads for all 4 batches and emits a partial
(pre-reduce) projection output; the host sums the 8 partials (the
tensor-parallel unshard).
"""

import numpy as np
from contextlib import ExitStack

import concourse.bass as bass
import concourse.tile as tile
from concourse import mybir
from concourse import bass_utils
from concourse.masks import make_identity

B, L, D = 4, 2048, 1024
NH, HS = 16, 64
NCORES = 8
HPC = 2                 # heads per core
CW = HPC * HS           # 128 head-cols per core
SCALE = 1.0 / 8.0       # 1/sqrt(HS)
F32 = mybir.dt.float32
BF16 = mybir.dt.float16  # fp16: 2-byte (xbar-transposable), 4 more mantissa bits than bf16
TOKS = B * L

# walrus in this toolchain rejects instructions carrying >1 sync-wait;
# move excess waits onto preceding same-engine NOPs.
def _split_excess_waits(nc, max_waits=1):
    for f in nc.m.functions:
        for blk in f.blocks:
            new_insts = []
            for inst in blk.instructions:
                si = getattr(inst, "sync_info", None)
                if si is not None and si.on_wait and len(si.on_wait) > max_waits:
                    waits = list(si.on_wait)
                    chunks = [waits[i:i + max_waits]
                              for i in range(0, len(waits), max_waits)]
                    for j, ch in enumerate(chunks[:-1]):
                        new_insts.append(mybir.InstNoOp(
                            name=f"{inst.name}-waitsplit{j}",
                            engine=inst.engine,
                            sync_info=mybir.SyncInfo(on_wait=ch, on_update=[]),
                            bass_nofuse=True,
                        ))
                    si.on_wait = chunks[-1]
                new_insts.append(inst)
            blk.instructions[:] = new_insts


def jb_min(ib):
    # U row-block ib (128 rows at i0=128*ib) needs Er-index columns
    # j >= 2047 - (i0+127); 512-wide column blocks from jb_min(ib) to 3.
    return max(0, (1920 - 128 * ib) // 512)


def build_program():
    nc = bass.Bass("TRN2", target_bir_lowering=False, debug=False,
                   num_devices=NCORES)
    xT = nc.declare_dram_parameter("xT", [D, TOKS], F32, isOutput=False)
    wqkv = nc.declare_dram_parameter("wqkv", [D, 3 * CW], F32, isOutput=False)
    bqkv = nc.declare_dram_parameter("bqkv", [3 * CW], F32, isOutput=False)
    ertd = nc.declare_dram_parameter("ertd", [2 * HS, L], F32, isOutput=False)
    wpa = nc.declare_dram_parameter("wpa", [HS, D], F32, isOutput=False)
    wpb = nc.declare_dram_parameter("wpb", [HS, D], F32, isOutput=False)
    bp = nc.declare_dram_parameter("bp", [D], F32, isOutput=False)
    part = nc.declare_dram_parameter("part", [TOKS, D], F32, isOutput=True)

    with tile.TileContext(nc) as tc, ExitStack() as ctx:
        singles = ctx.enter_context(tc.tile_pool(name="singles", bufs=1))
        pb = ctx.enter_context(tc.tile_pool(name="perbatch", bufs=2))
        xin = ctx.enter_context(tc.tile_pool(name="xin", bufs=2))
        work = ctx.enter_context(tc.tile_pool(name="work", bufs=3))
        workb = ctx.enter_context(tc.tile_pool(name="workb", bufs=4))
        udram = ctx.enter_context(tc.tile_pool(name="udram", bufs=4, space="DRAM"))
        ps_mm = ctx.enter_context(tc.tile_pool(name="ps_mm", bufs=2, space="PSUM"))
        ps_s = ctx.enter_context(tc.tile_pool(name="ps_s", bufs=2, space="PSUM"))
        ps_y = ctx.enter_context(tc.tile_pool(name="ps_y", bufs=2, space="PSUM"))
        ps_x = ctx.enter_context(tc.tile_pool(name="ps_x", bufs=2, space="PSUM"))

        # ---- constants / weights ----
        w_sb = singles.tile([128, 8 * 3 * CW], F32)
        nc.sync.dma_start(w_sb[:], wqkv.ap().rearrange(
            "(kb p) m -> kb p m", p=128).transpose([1, 0, 2]))
        bq_row = singles.tile([1, 3 * CW], F32)
        nc.sync.dma_start(bq_row[:], bqkv.ap().unsqueeze(0))
        ertd_sb = singles.tile([128, L], F32)
        nc.sync.dma_start(ertd_sb[:], ertd.ap())
        wpa_sb = singles.tile([HS, D], F32)
        nc.sync.dma_start(wpa_sb[:], wpa.ap())
        wpb_sb = singles.tile([HS, D], F32)
        nc.sync.dma_start(wpb_sb[:], wpb.ap())
        bp_row = singles.tile([1, D], F32)
        nc.sync.dma_start(bp_row[:], bp.ap().unsqueeze(0))
        ones_row = singles.tile([1, 512], F32)
        nc.vector.memset(ones_row[:], 1.0)
        ident = singles.tile([128, 128], F32)
        make_identity(nc, ident[:])
        neg_fill = nc.gpsimd.to_reg(-1e30)

        NT = L // 128          # 16 token-blocks per batch
        for b in range(B):
            # ---------- qkv^T = [q;k;v] cols x tokens ----------
            qT = pb.tile([128, L], F32, tag="qT")
            kT = pb.tile([128, L], F32, tag="kT")
            va = pb.tile([128, NT * (HS + 1)], F32, tag="va")
            vb = pb.tile([128, NT * (HS + 1)], F32, tag="vb")
            nc.gpsimd.memset(va[:], 1.0)
            nc.gpsimd.memset(vb[:], 1.0)
            for tch in range(4):        # 512-token chunks
                col0 = b * L + tch * 512
                xchunk = xin.tile([128, 8 * 512], F32, tag="xchunk")
                nc.sync.dma_start(
                    xchunk[:],
                    xT.ap()[:, col0:col0 + 512].rearrange(
                        "(kb p) n -> kb p n", p=128).transpose([1, 0, 2]))
                for m in range(3):      # q, k, v col-groups of 128
                    ps = ps_mm.tile([128, 512], F32, tag="mm")
                    nc.tensor.matmul(ps[:], bq_row[0:1, m * 128:(m + 1) * 128],
                                     ones_row[0:1, :], start=True, stop=False)
                    for kb in range(8):
                        nc.tensor.matmul(
                            ps[:],
                            w_sb[:, kb * 384 + m * 128: kb * 384 + (m + 1) * 128],
                            xchunk[:, kb * 512:(kb + 1) * 512],
                            start=False, stop=(kb == 7))
                    if m == 0:
                        nc.scalar.activation(qT[:, tch * 512:(tch + 1) * 512], ps[:],
                                             mybir.ActivationFunctionType.Copy)
                    elif m == 1:
                        nc.scalar.activation(kT[:, tch * 512:(tch + 1) * 512], ps[:],
                                             mybir.ActivationFunctionType.Copy)
                    else:
                        vtmp = work.tile([128, 512], F32, tag="vtmp")
                        nc.vector.tensor_copy(vtmp[:], ps[:])
                        for s in range(4):
                            tk = tch * 4 + s
                            pt = ps_x.tile([128, 128], F32, tag="x")
                            nc.tensor.transpose(pt[:], vtmp[:, s * 128:(s + 1) * 128],
                                                ident[:])
                            nc.vector.tensor_copy(
                                va[:, tk * (HS + 1): tk * (HS + 1) + HS], pt[:, 0:HS])
                            nc.vector.tensor_copy(
                                vb[:, tk * (HS + 1): tk * (HS + 1) + HS], pt[:, HS:2 * HS])

            # ---------- U = scale * (q . Er^T) per head -> DRAM bf16 ----------
            u_a = udram.tile([L, L], BF16, tag="u_a")
            u_b = udram.tile([L, L], BF16, tag="u_b")
            for ib in range(NT):
                for jb in range(jb_min(ib), 4):
                    pua = ps_mm.tile([128, 512], F32, tag="mm")
                    pub = ps_mm.tile([128, 512], F32, tag="mm")
                    nc.tensor.matmul(pua[:], qT[0:HS, ib * 128:(ib + 1) * 128],
                                     ertd_sb[0:HS, jb * 512:(jb + 1) * 512],
                                     start=True, stop=True)
                    nc.tensor.matmul(pub[:], qT[HS:128, ib * 128:(ib + 1) * 128],
                                     ertd_sb[HS:128, jb * 512:(jb + 1) * 512],
                                     start=True, stop=True)
                    ua_bf = workb.tile([128, 512], BF16, tag="ubf")
                    ub_bf = workb.tile([128, 512], BF16, tag="ubf")
                    nc.vector.tensor_scalar_mul(ua_bf[:], pua[:], SCALE)
                    nc.scalar.activation(ub_bf[:], pub[:],
                                         mybir.ActivationFunctionType.Copy, scale=SCALE)
                    nc.sync.dma_start(
                        u_a[ib * 128:(ib + 1) * 128, jb * 512:(jb + 1) * 512], ua_bf[:])
                    nc.sync.dma_start(
                        u_b[ib * 128:(ib + 1) * 128, jb * 512:(jb + 1) * 512], ub_bf[:])

            # ---------- attention per 512-query block, heads packed ----------
            yna = pb.tile([HS, L], F32, tag="yna")
            ynb = pb.tile([HS, L], F32, tag="ynb")
            for hb, (u_t, v_t, yn) in enumerate(((u_a, va, yna), (u_b, vb, ynb))):
                hlo = hb * HS        # head row offset in qT/kT
                for ib5 in range(4):
                    i0 = ib5 * 512
                    py = ps_y.tile([HS + 1, 512], F32, tag="y")
                    n_mb = 4 * (ib5 + 1)
                    for mb in range(n_mb):
                        m0 = mb * 128
                        ss = ps_s.tile([128, 512], F32, tag="s")
                        nc.tensor.matmul(ss[:], kT[hlo:hlo + HS, m0:m0 + 128],
                                         qT[hlo:hlo + HS, i0:i0 + 512],
                                         start=True, stop=True)
                        uap = u_t[:]
                        srel = workb.tile([128, 512], BF16, tag="srel")
                        nc.sync.dma_start_transpose(
                            srel[:],
                            bass.AP(uap.tensor,
                                    uap.offset + i0 * (L - 1) + (L - 1) + m0,
                                    [[L - 1, 512], [1, 128]]))
                        ssum = work.tile([128, 512], F32, tag="ssum")
                        nc.vector.scalar_tensor_tensor(
                            ssum[:], ss[:], SCALE, srel[:],
                            op0=mybir.AluOpType.mult, op1=mybir.AluOpType.add)
                        if m0 >= i0:   # diagonal block: causal mask
                            nc.gpsimd.affine_select(
                                out=ssum[:], in_=ssum[:],
                                compare_op=mybir.AluOpType.is_ge,
                                fill=neg_fill, base=i0 - m0, channel_multiplier=-1,
                                pattern=[[1, 512]])
                        et = work.tile([128, 512], F32, tag="et")
                        nc.scalar.activation(et[:], ssum[:],
                                             mybir.ActivationFunctionType.Exp)
                        nc.tensor.matmul(py[:],
                                         v_t[:, mb * (HS + 1):(mb + 1) * (HS + 1)],
                                         et[:],
                                         start=(mb == 0), stop=(mb == n_mb - 1))
                    recip = work.tile([1, 512], F32, tag="recip")
                    nc.vector.reciprocal(recip[:], py[HS:HS + 1, :])
                    pbc = ps_x.tile([HS, 512], F32, tag="x")
                    nc.tensor.matmul(pbc[:], ones_row[0:1, 0:HS], recip[:],
                                     start=True, stop=True)
                    bc_sb = work.tile([HS, 512], F32, tag="bcsb")
                    nc.scalar.activation(bc_sb[:], pbc[:],
                                         mybir.ActivationFunctionType.Copy)
                    nc.vector.tensor_mul(yn[:, i0:i0 + 512], py[0:HS, :], bc_sb[:])

            # ---------- partial projection ----------
            for tk in range(NT):
                t0 = tk * 128
                for nb in range(2):
                    po = ps_mm.tile([128, 512], F32, tag="mm")
                    nc.tensor.matmul(po[:], ones_row[0:1, 0:128],
                                     bp_row[0:1, nb * 512:(nb + 1) * 512],
                                     start=True, stop=False)
                    nc.tensor.matmul(po[:], yna[:, t0:t0 + 128],
                                     wpa_sb[:, nb * 512:(nb + 1) * 512],
                                     start=False, stop=False)
                    nc.tensor.matmul(po[:], ynb[:, t0:t0 + 128],
                                     wpb_sb[:, nb * 512:(nb + 1) * 512],
                                     start=False, stop=True)
                    osb = work.tile([128, 512], F32, tag="osb")
                    if nb == 0:
                        nc.scalar.activation(osb[:], po[:],
                                             mybir.ActivationFunctionType.Copy)
                    else:
                        nc.vector.tensor_copy(osb[:], po[:])
                    nc.sync.dma_start(
                        part.ap()[b * L + t0: b * L + t0 + 128,
                                  nb * 512:(nb + 1) * 512], osb[:])

    return nc


def make_in_maps(x, W_attn, b_attn, Er, W_proj, b_proj):
    x = np.asarray(x, np.float32)
    W_attn = np.asarray(W_attn, np.float32)
    b_attn = np.asarray(b_attn, np.float32)
    Er = np.asarray(Er, np.float32)
    W_proj = np.asarray(W_proj, np.float32)
    b_proj = np.asarray(b_proj, np.float32)
    xT = np.ascontiguousarray(x.reshape(TOKS, D).T)
    ErT = np.ascontiguousarray(Er.T)
    ertd = np.concatenate([ErT, ErT], axis=0)
    zeros_bp = np.zeros_like(b_proj)
    in_maps = []
    for c in range(NCORES):
        q0 = CW * c
        wq = W_attn[:, q0:q0 + CW]
        wk = W_attn[:, D + q0:D + q0 + CW]
        wv = W_attn[:, 2 * D + q0:2 * D + q0 + CW]
        in_maps.append(dict(
            xT=xT,
            wqkv=np.ascontiguousarray(np.concatenate([wq, wk, wv], axis=1)),
            bqkv=np.ascontiguousarray(np.concatenate(
                [b_attn[q0:q0 + CW], b_attn[D + q0:D + q0 + CW],
                 b_attn[2 * D + q0:2 * D + q0 + CW]])),
            ertd=ertd,
            wpa=np.ascontiguousarray(W_proj[q0:q0 + HS, :]),
            wpb=np.ascontiguousarray(W_proj[q0 + HS:q0 + CW, :]),
            bp=b_proj if c == 0 else zeros_bp,
        ))
    return in_maps


_cached_nc = None


def kernel(x, W_attn, b_attn, Er, W_proj, b_proj):
    global _cached_nc
    if _cached_nc is None:
        _cached_nc = build_program()
        _split_excess_waits(_cached_nc)
    nc = _cached_nc
    in_maps = make_in_maps(x, W_attn, b_attn, Er, W_proj, b_proj)
    res = bass_utils.run_bass_kernel_spmd(nc, in_maps, list(range(NCORES)))
    out = np.zeros((TOKS, D), np.float32)
    for c in range(NCORES):
        out += res.results[c]["part"]
    return out.reshape(B, L, D)
